# revision 8
# baseline (speedup 1.0000x reference)
"""Trainium2 Bass kernel for nn_DiagonalTraining (ragged per-anti-diagonal linear).

Math (reference): for each batch image x[b] (SxS) and each anti-diagonal
i (elements x[b, r, i-r], r=0..i), apply a per-diagonal linear layer:
  out[b,i,q] = sum_{r<=i} x[b,r,i-r] * W[i,q,r] + bias[i,q]   (q <= i)
and scatter back: y[b,q,i-q] = out[b,i,q]; positions with r+c >= S keep x.

Distribution: diagonal i -> core i%8, slot j=i//8 (64 slots per core,
balanced by construction). Host packs, per (core, slot), two operand
planes whose rows are the contraction axis r (zero-padded to the
core-independent NJ=8*(j+1) so the SPMD program is identical on all
cores):
  D^T[r,b] = x[b,r,i-r]        (bf16, 32 cols)
  V[r,q]   = 32 * W[i,q,r]     (fp8 e3m4, NJ cols; x32 keeps the
                                0.05-scaled weights in e3m4's normal
                                range [0.25,15.5]; descaled on host)
Mixed-precision halves the dominant HBM traffic (V is ~85% of bytes)
while keeping max rel-err ~1.3e-2 (< 2e-2 gate): the fp8 noise lives
only on W; D stays bf16. The per-diagonal bias is added on the host
while scattering results back (elementwise, ~0.05% of the FLOPs).

Device: slots are split into row-chunks (contraction tiles): full
128-row units plus a remainder unit padded to 64 rows when the live
remainder fits (else 128) -- "pad64". 64-row units of neighbouring
slots pair into one 128-row strip at partition offsets {0,64}. Each
strip occupies a V-window column range (width NJ) and a D-window range
(width 32) at the same partition offsets. All windows are SBUF
resident; loads are issued up front in consumption order, D windows
interleaved just before the first V window that needs them. Matmuls
(lhsT = D^T chunk [rows,32] bf16, rhs = V chunk [rows,NJ] fp8)
accumulate psum[32, NJ] per slot inside a bank-packed 4-slot group
psum tile; per group one copy (alternating vector/scalar engines)
downcasts psum->fp16 stage (fp16 keeps output quantization noise at
~5e-4, preserving the fp8 error budget) and a store DMA writes it out,
overlapping remaining loads.
"""

import sys

for _p in ("/opt/trn_rl_repo", "/opt/pypackages"):
    if _p not in sys.path:
        sys.path.append(_p)

import numpy as np

import concourse.bass as bass  # noqa: F401
import concourse.tile as tile
from concourse import bacc, mybir
from concourse.bass_utils import run_bass_kernel_spmd

B = 32          # batch
S = 512         # seq len / number of diagonals
N_CORES = 8
N_SLOTS = S // N_CORES  # 64 slots per core
DCOL = B        # width of the D^T block (matmul M axis)
GROUP = 4       # slots per psum group
N_GROUPS = N_SLOTS // GROUP
W_SCALE = 32.0  # host-side V prescale for e3m4 range
F8_MAX = 15.5   # e3m4 max finite

KCFG = {
    "v_dt": "e3m4",      # V blob dtype
    "d_dt": "bf16",      # D blob dtype
    "out_dt": "f16",     # output blob dtype
    "psum_bufs": 2,
    "stage_bufs": 4,
    "copy_engines": ("vector", "scalar"),
    "store_engine": "sync",
    "v_queue": "gpsimd",
    "d_queue": "gpsimd",
    "pack": "pad128",    # pad64 tile_position=(64,0) crashes the PE; pad128 only
    # head loads on the two HWDGE queues: SWDGE Q7 only starts draining
    # descriptors ~7us in (engine preamble), so the first windows go via
    # scalar/sync to feed the PE from ~2us
    "head_hwdge": True,
    "v_head_sync": 3,    # V windows 1..N on sync (V0 + D0 on scalar)
    # PE clock warmup: HAM gates the PE to 1.2GHz until ~3.4us of
    # sustained activity; N zero matmuls starting at t~0 open the gate
    # before the first real matmul arrives
    "warm_mm": 8,
    "tail_merge": 4,     # last N groups share one stage tile + store
}

# test-only bisection hook (harness never sets this)
import os as _os  # noqa: E402

if _os.environ.get("KCFG_OVERRIDE"):
    import json as _json

    KCFG.update(_json.loads(_os.environ["KCFG_OVERRIDE"]))

# ---- static layout ----------------------------------------------------
_ORDER = list(range(N_SLOTS - 1, -1, -1))   # largest slot first
_GROUPS = [_ORDER[g * GROUP : (g + 1) * GROUP] for g in range(N_GROUPS)]


def _build_units():
    """Per slot: list of (row_start, unit_rows). PE base partitions allow
    pb in {0,64} for 64-row operands, so remainder units are 64 or 128."""
    units = {}
    for j in range(N_SLOTS):
        NJ = 8 * (j + 1)
        f = NJ // 128
        rem = NJ - 128 * f
        us = [(128 * c, 128) for c in range(f)]
        if rem:
            if KCFG["pack"] == "pad64" and rem <= 64:
                us.append((128 * f, 64))
            else:
                us.append((128 * f, 128))
        units[j] = us
    return units


def _build_strips():
    """Stack units into 128-row strips: [(j, row_start, rows, pb)].

    Full units get their own strip. 64-row units pair with the
    neighbouring such slot (within a 16-slot band, descending j) at
    partition offsets 0/64; widths differ by <=56 cols so mismatch
    waste stays tiny. Strips are emitted largest slot first.
    """
    units = _build_units()
    halves = {}
    for j in range(N_SLOTS):
        for rs, rows in units[j]:
            if rows == 64:
                halves[j] = rs
    half_groups = {}
    for t in range(4):
        hs = [j for j in range(16 * t + 15, 16 * t - 1, -1) if j in halves]
        for a in range(0, len(hs) - 1, 2):
            half_groups[hs[a]] = hs[a : a + 2]
        if len(hs) % 2:
            half_groups[hs[-1]] = [hs[-1]]
    strips = []
    done = set()
    for j in _ORDER:
        for rs, rows in units[j]:
            if rows == 128:
                strips.append([(j, rs, 128, 0)])
            else:
                if j in done:
                    continue
                grp = half_groups.get(j)
                if grp is None:
                    continue
                mem = []
                pb = 0
                for p in grp:
                    mem.append((p, halves[p], 64, pb))
                    pb += 64
                strips.append(mem)
                done.update(grp)
    return strips


def _wcapv(w):
    # small first windows so the first matmuls start early
    return (1024, 2048)[w] if w < 2 else 3072


D_WIN_CAP = 1024  # D window cols (32 strips each)

# chunk placement: j -> list of (vwin, vcb, dwin, dcb, pb, rows, row_start)
_SLOT_CHUNKS = {j: [] for j in range(N_SLOTS)}
_VWIN_W = []
_DWIN_W = []
_STRIP_DWIN = []   # strip idx -> d window idx
_cur_vw, _cur_vc = 0, 0
_cur_dw, _cur_dc = 0, 0
_STRIPS = _build_strips()
_VW_NEED_D = {}    # v window -> max d window required
for _si, _members in enumerate(_STRIPS):
    _sw = max(8 * (_j + 1) for _j, _, _, _ in _members)
    if _cur_vc + _sw > _wcapv(_cur_vw):
        _VWIN_W.append(_cur_vc)
        _cur_vw += 1
        _cur_vc = 0
    if _cur_dc + 32 > D_WIN_CAP:
        _DWIN_W.append(_cur_dc)
        _cur_dw += 1
        _cur_dc = 0
    for _j, _rs, _rows, _pb in _members:
        _SLOT_CHUNKS[_j].append(
            (_cur_vw, _cur_vc, _cur_dw, _cur_dc, _pb, _rows, _rs)
        )
    _VW_NEED_D[_cur_vw] = _cur_dw
    _cur_vc += _sw
    _cur_dc += 32
_VWIN_W.append(_cur_vc)
_DWIN_W.append(_cur_dc)
N_VWINS = len(_VWIN_W)
N_DWINS = len(_DWIN_W)
for _j in range(N_SLOTS):
    _SLOT_CHUNKS[_j].sort(key=lambda c: c[6])

_VWIN_OFF = []
_boff = 0
for _w in range(N_VWINS):
    _VWIN_OFF.append(_boff)
    _boff += 128 * _VWIN_W[_w]
VBLOB_ELEMS = _boff
_DWIN_OFF = []
_boff = 0
for _w in range(N_DWINS):
    _DWIN_OFF.append(_boff)
    _boff += 128 * _DWIN_W[_w]
DBLOB_ELEMS = _boff

# psum group column layout (bank-aligned, no matmul straddles a bank).
# The psum->stage copies compact the bank-alignment gaps away, so the
# stage/store/output layout uses gap-free "compact" columns.
_BANK = 512
_GROUP_COLS = []    # g -> [(j, psum_col)]
_GROUP_W = []       # g -> psum tile width (with gaps)
_GROUP_RANGES = []  # g -> [(psum_off, compact_off, width)] copy ranges
_GROUP_CW = []      # g -> compact width
for _slots in _GROUPS:
    _col = 0
    _ccol = 0
    _cols = []
    _ranges = []
    for _j in _slots:
        _NJ = 8 * (_j + 1)
        if _col // _BANK != (_col + _NJ - 1) // _BANK:
            _col = ((_col + _BANK - 1) // _BANK) * _BANK
        if _ranges and _ranges[-1][0] + _ranges[-1][2] == _col:
            _ranges[-1] = (_ranges[-1][0], _ranges[-1][1], _ranges[-1][2] + _NJ)
        else:
            _ranges.append((_col, _ccol, _NJ))
        _cols.append((_j, _col))
        _col += _NJ
        _ccol += _NJ
    _GROUP_COLS.append(_cols)
    _GROUP_W.append(_col)
    _GROUP_RANGES.append(_ranges)
    _GROUP_CW.append(_ccol)

# store groups: the last tail_merge groups share one stage tile/store
# (their per-group stores are tiny and each pays ~1-2us of DMA
# completion latency; one batched store trims the kernel tail)
_TM = max(1, KCFG["tail_merge"])
_SGROUPS = [[g] for g in range(N_GROUPS - _TM)] + [
    list(range(N_GROUPS - _TM, N_GROUPS))
]
_SG_OF_GROUP = {}   # g -> (sg index, col offset of g inside sg stage)
_SG_CW = []
_SG_OFF = []
_goff = 0
for _si, _gl in enumerate(_SGROUPS):
    _off = 0
    for _g in _gl:
        _SG_OF_GROUP[_g] = (_si, _off)
        _off += _GROUP_CW[_g]
    _SG_CW.append(_off)
    _SG_OFF.append(_goff)
    _goff += B * _off
OUT_ELEMS = _goff

_SLOT_OUT = {}      # j -> (sg, compact col within sg stage)
for _g in range(N_GROUPS):
    _si, _goffc = _SG_OF_GROUP[_g]
    _ccol = 0
    for _j, _col in _GROUP_COLS[_g]:
        _SLOT_OUT[_j] = (_si, _goffc + _ccol)
        _ccol += 8 * (_j + 1)

# load issue order: D windows interleave just before first use
_LOAD_ORDER = []
_d_done = -1
for _w in range(N_VWINS):
    while _d_done < _VW_NEED_D.get(_w, -1):
        _d_done += 1
        _LOAD_ORDER.append(("d", _d_done))
    _LOAD_ORDER.append(("v", _w))
while _d_done < N_DWINS - 1:
    _d_done += 1
    _LOAD_ORDER.append(("d", _d_done))

_compiled_nc = None


def _dt(name):
    return {
        "f32": mybir.dt.float32,
        "f32r": mybir.dt.float32r,
        "bf16": mybir.dt.bfloat16,
        "f16": mybir.dt.float16,
        "e3m4": mybir.dt.float8e3,
        "e4m3": mybir.dt.float8e4,
    }[name]


def _build_program():
    global _compiled_nc
    if _compiled_nc is not None:
        return _compiled_nc

    from contextlib import ExitStack

    nc = bacc.Bacc("TRN2", target_bir_lowering=False, debug=False)
    f32 = mybir.dt.float32
    v_dt = _dt(KCFG["v_dt"])
    d_dt = _dt(KCFG["d_dt"])
    out_dt = _dt(KCFG["out_dt"])
    vblob = nc.dram_tensor("vblob", [VBLOB_ELEMS], v_dt, kind="ExternalInput").ap()
    dblob = nc.dram_tensor("dblob", [DBLOB_ELEMS], d_dt, kind="ExternalInput").ap()
    outb = nc.dram_tensor("outblob", [OUT_ELEMS], out_dt, kind="ExternalOutput").ap()

    store_eng = getattr(nc, KCFG["store_engine"])
    copy_engs = [getattr(nc, e) for e in KCFG["copy_engines"]]
    v_eng = getattr(nc, KCFG["v_queue"])
    d_eng = getattr(nc, KCFG["d_queue"])

    def _load_eng(kind, idx):
        if not KCFG["head_hwdge"]:
            return v_eng if kind == "v" else d_eng
        if kind == "d":
            return nc.scalar if idx == 0 else d_eng
        if idx == 0:
            return nc.scalar
        if idx <= KCFG["v_head_sync"]:
            return nc.sync
        return v_eng

    with tile.TileContext(nc) as tc, ExitStack() as ctx:
        win_pool = ctx.enter_context(tc.tile_pool(name="win", bufs=1))
        stage_pool = ctx.enter_context(
            tc.tile_pool(name="stage", bufs=KCFG["stage_bufs"])
        )
        psum_pool = ctx.enter_context(
            tc.tile_pool(name="psum", bufs=KCFG["psum_bufs"], space="PSUM")
        )

        # PE clock warmup: zero matmuls into a throwaway psum tile
        if KCFG["warm_mm"]:
            warm_pool = ctx.enter_context(tc.tile_pool(name="warm", bufs=1))
            wd = warm_pool.tile([128, DCOL], d_dt, name="warm_d", tag="warm_d")
            wv = warm_pool.tile([128, 512], v_dt, name="warm_v", tag="warm_v")
            nc.vector.memset(wd[:], 0)
            nc.vector.memset(wv[:], 0)
            wp = psum_pool.tile([B, 512], f32, name="warm_p", tag="psum")
            for _ in range(KCFG["warm_mm"]):
                nc.tensor.matmul(
                    wp[:], wd[:], wv[:], start=True, stop=True
                )

        vwin_tiles = [None] * N_VWINS
        dwin_tiles = [None] * N_DWINS
        for kind, w in _LOAD_ORDER:
            if kind == "v":
                wf = _VWIN_W[w]
                t = win_pool.tile([128, wf], v_dt, name=f"vw{w}", tag=f"vw{w}")
                src = vblob[_VWIN_OFF[w] : _VWIN_OFF[w] + 128 * wf].rearrange(
                    "(p f) -> p f", p=128, f=wf
                )
                _load_eng("v", w).dma_start(t[:], src)
                vwin_tiles[w] = t
            else:
                wf = _DWIN_W[w]
                t = win_pool.tile([128, wf], d_dt, name=f"dw{w}", tag=f"dw{w}")
                src = dblob[_DWIN_OFF[w] : _DWIN_OFF[w] + 128 * wf].rearrange(
                    "(p f) -> p f", p=128, f=wf
                )
                _load_eng("d", w).dma_start(t[:], src)
                dwin_tiles[w] = t

        def _copy(eng, dst_ap, src_ap):
            if eng is nc.scalar:
                eng.copy(dst_ap, src_ap)
            else:
                eng.tensor_copy(dst_ap, src_ap)

        for si, gl in enumerate(_SGROUPS):
            scw = _SG_CW[si]
            stage_t = stage_pool.tile(
                [B, scw], out_dt, name=f"st{si}", tag="stage"
            )
            for g in gl:
                slots = _GROUPS[g]
                gw = _GROUP_W[g]
                gcol = _SG_OF_GROUP[g][1]
                psum_t = psum_pool.tile([B, gw], f32, name=f"psum{g}", tag="psum")
                for j, col in _GROUP_COLS[g]:
                    NJ = 8 * (j + 1)
                    chs = _SLOT_CHUNKS[j]
                    for c, (vw, vcb, dw, dcb, pb, rows, _rs) in enumerate(chs):
                        vt = vwin_tiles[vw]
                        dt_ = dwin_tiles[dw]
                        nc.tensor.matmul(
                            psum_t[:, col : col + NJ],
                            dt_[pb : pb + rows, dcb : dcb + DCOL],
                            vt[pb : pb + rows, vcb : vcb + NJ],
                            start=(c == 0),
                            stop=(c == len(chs) - 1),
                            tile_position=None if pb == 0 else (pb, 0),
                        )
                ceng = copy_engs[g % len(copy_engs)]
                for po, co, wdt in _GROUP_RANGES[g]:
                    _copy(
                        ceng,
                        stage_t[:, gcol + co : gcol + co + wdt],
                        psum_t[:, po : po + wdt],
                    )
            dst = outb[_SG_OFF[si] : _SG_OFF[si] + B * scw].rearrange(
                "(p w) -> p w", p=B, w=scw
            )
            store_eng.dma_start(dst, stage_t[:])

    nc.compile()
    _compiled_nc = nc
    return nc


def _np_dt(name):
    import ml_dtypes

    return {
        "bf16": ml_dtypes.bfloat16,
        "f16": np.float16,
        "f32": np.float32,
        "e3m4": ml_dtypes.float8_e3m4,
        "e4m3": ml_dtypes.float8_e4m3,
    }[name]


def _pack_core(k, x, W):
    vnp = _np_dt(KCFG["v_dt"])
    dnp = _np_dt(KCFG["d_dt"])
    vblob = np.zeros(VBLOB_ELEMS, vnp)
    dblob = np.zeros(DBLOB_ELEMS, dnp)
    for j in range(N_SLOTS):
        i = N_CORES * j + k
        ni = i + 1
        NJ = 8 * (j + 1)
        r = np.arange(ni)
        Dt = np.zeros((NJ, DCOL), np.float32)
        Dt[:ni] = x[:, r, i - r].T                       # D^T[r, b]
        V = np.zeros((NJ, NJ), np.float32)
        V[:ni, :ni] = np.clip(W[i, :ni, :ni].T * W_SCALE, -F8_MAX, F8_MAX)
        for vw, vcb, dw, dcb, pb, rows, rs in _SLOT_CHUNKS[j]:
            vimg = vblob[_VWIN_OFF[vw] : _VWIN_OFF[vw] + 128 * _VWIN_W[vw]]
            vimg = vimg.reshape(128, _VWIN_W[vw])
            vrl = V[rs : rs + rows]          # may be shorter than rows
            vimg[pb : pb + vrl.shape[0], vcb : vcb + NJ] = vrl.astype(vnp)
            dimg = dblob[_DWIN_OFF[dw] : _DWIN_OFF[dw] + 128 * _DWIN_W[dw]]
            dimg = dimg.reshape(128, _DWIN_W[dw])
            drl = Dt[rs : rs + rows]
            dimg[pb : pb + drl.shape[0], dcb : dcb + DCOL] = drl.astype(dnp)
    return vblob, dblob


def kernel(x, W, b):
    x = np.asarray(x, np.float32)
    W = np.asarray(W, np.float32)
    b = np.asarray(b, np.float32)

    nc = _build_program()
    in_maps = []
    for k in range(N_CORES):
        vb, db = _pack_core(k, x, W)
        in_maps.append({"vblob": vb, "dblob": db})
    res = run_bass_kernel_spmd(nc, in_maps, list(range(N_CORES)))

    y = x.copy()
    inv = 1.0 / W_SCALE
    for k in range(N_CORES):
        ob = res.results[k]["outblob"]
        for j in range(N_SLOTS):
            i = N_CORES * j + k
            ni = i + 1
            si, col = _SLOT_OUT[j]
            scw = _SG_CW[si]
            blk = np.asarray(
                ob[_SG_OFF[si] : _SG_OFF[si] + B * scw], np.float32
            ).reshape(B, scw)
            q = np.arange(ni)
            y[:, q, i - q] = blk[:, col : col + ni] * inv + b[i, :ni][None]
    return y


def emulate(x, W, b):
    """Exact host emulation of the device numeric path (for testing)."""
    x = np.asarray(x, np.float32)
    W = np.asarray(W, np.float32)
    b = np.asarray(b, np.float32)
    out_np = _np_dt(KCFG["out_dt"])
    y = x.copy()
    inv = 1.0 / W_SCALE
    for k in range(N_CORES):
        vb, db = _pack_core(k, x, W)
        for j in range(N_SLOTS):
            i = N_CORES * j + k
            ni = i + 1
            NJ = 8 * (j + 1)
            acc = np.zeros((B, NJ), np.float32)
            for vw, vcb, dw, dcb, pb, rows, _rs in _SLOT_CHUNKS[j]:
                vimg = vb[_VWIN_OFF[vw] : _VWIN_OFF[vw] + 128 * _VWIN_W[vw]]
                vimg = vimg.reshape(128, _VWIN_W[vw])
                dimg = db[_DWIN_OFF[dw] : _DWIN_OFF[dw] + 128 * _DWIN_W[dw]]
                dimg = dimg.reshape(128, _DWIN_W[dw])
                vch = vimg[pb : pb + rows, vcb : vcb + NJ].astype(np.float32)
                dch = dimg[pb : pb + rows, dcb : dcb + DCOL].astype(np.float32)
                acc += dch.T @ vch
            stg = acc.astype(out_np).astype(np.float32)
            q = np.arange(ni)
            y[:, q, i - q] = stg[:, :ni] * inv + b[i, :ni][None]
    return y


if __name__ == "__main__":
    vb = VBLOB_ELEMS * {"e3m4": 1, "e4m3": 1, "bf16": 2, "f32": 4}[KCFG["v_dt"]]
    db = DBLOB_ELEMS * {"bf16": 2, "f32": 4}[KCFG["d_dt"]]
    ob = OUT_ELEMS * {"f16": 2, "bf16": 2, "f32": 4}[KCFG["out_dt"]]
    print(f"V windows: {N_VWINS} ({_VWIN_W}), bytes {vb}")
    print(f"D windows: {N_DWINS} ({_DWIN_W}), bytes {db}")
    print(f"out bytes {ob}; total DMA {(vb + db + ob) / 1e6:.2f} MB")
    print(f"PE cols {sum(NJ for j in range(N_SLOTS) for NJ in [8 * (j + 1)] for _ in _SLOT_CHUNKS[j])}")


# revision 13
# speedup vs baseline: 1.2486x; 1.2486x over previous
"""Trainium2 Bass kernel for nn_DiagonalTraining (ragged per-anti-diagonal linear).

Math (reference): for each batch image x[b] (SxS) and each anti-diagonal
i (elements x[b, r, i-r], r=0..i), apply a per-diagonal linear layer:
  out[b,i,q] = sum_{r<=i} x[b,r,i-r] * W[i,q,r] + bias[i,q]   (q <= i)
and scatter back: y[b,q,i-q] = out[b,i,q]; positions with r+c >= S keep x.

Distribution: diagonal i -> core i%8, slot j=i//8 (64 slots per core,
balanced by construction). Host packs, per (core, slot), two operand
planes whose rows are the contraction axis r (zero-padded to the
core-independent NJ=8*(j+1) so the SPMD program is identical on all
cores):
  D^T[r,b] = x[b,r,i-r]        (bf16, 32 cols)
  V[r,q]   = 32 * W[i,q,r]     (fp8 e3m4, NJ cols; x32 keeps the
                                0.05-scaled weights in e3m4's normal
                                range [0.25,15.5]; descaled on host)
Mixed-precision halves the dominant HBM traffic (V is ~85% of bytes)
while keeping max rel-err ~1.3e-2 (< 2e-2 gate): the fp8 noise lives
only on W; D stays bf16. The per-diagonal bias is added on the host
while scattering results back (elementwise, ~0.05% of the FLOPs).

Device: slots are split into row-chunks (contraction tiles): full
128-row units plus a remainder unit padded to 64 rows when the live
remainder fits (else 128) -- "pad64". 64-row units of neighbouring
slots pair into one 128-row strip at partition offsets {0,64}. Each
strip occupies a V-window column range (width NJ) and a D-window range
(width 32) at the same partition offsets. All windows are SBUF
resident; loads are issued up front in consumption order, D windows
interleaved just before the first V window that needs them. Matmuls
(lhsT = D^T chunk [rows,32] bf16, rhs = V chunk [rows,NJ] fp8)
accumulate psum[32, NJ] per slot inside a bank-packed 4-slot group
psum tile; per group one copy (alternating vector/scalar engines)
downcasts psum->fp16 stage (fp16 keeps output quantization noise at
~5e-4, preserving the fp8 error budget) and a store DMA writes it out,
overlapping remaining loads.
"""

import sys

for _p in ("/opt/trn_rl_repo", "/opt/pypackages"):
    if _p not in sys.path:
        sys.path.append(_p)

import numpy as np

import concourse.bass as bass  # noqa: F401
import concourse.tile as tile
from concourse import bacc, mybir
from concourse.bass_utils import run_bass_kernel_spmd

B = 32          # batch
S = 512         # seq len / number of diagonals
N_CORES = 8
N_SLOTS = S // N_CORES  # 64 slots per core
DCOL = B        # width of the D^T block (matmul M axis)
GROUP = 4       # slots per psum group
N_GROUPS = N_SLOTS // GROUP
W_SCALE = 32.0  # host-side V prescale for e3m4 range
F8_MAX = 15.5   # e3m4 max finite

KCFG = {
    "v_dt": "e3m4",      # V blob dtype
    "d_dt": "bf16",      # D blob dtype
    "out_dt": "f16",     # output blob dtype
    "psum_bufs": 2,
    "stage_bufs": 4,
    "copy_engines": ("vector", "scalar"),
    # alternate store queues: one HWDGE ring serializes the ~1-2us HBM
    # completion receipt per store, pacing the whole back half
    "store_engines": ("sync", "scalar"),
    "v_queue": "gpsimd",
    "d_queue": "gpsimd",
    "pack": "pad128",    # pad64 tile_position=(64,0) crashes the PE; pad128 only
    # NOTE (measured): every engine has a ~7-9us init preamble (TENSOR_LOAD,
    # ACT tables, sem chains) before user work issues — HWDGE "head" loads
    # and PE-warmup matmuls cannot beat the SWDGE queue to first byte and
    # only delay the stream; keep all loads on the gpsimd SWDGE queue.
    "head_hwdge": False,
    "v_head_sync": 0,
    "warm_mm": 0,
    # groups per store-group: merged stages cut the store count (and the
    # per-store completion round trips) in half
    "store_group": 2,
}

# test-only bisection hook (harness never sets this)
import os as _os  # noqa: E402

if _os.environ.get("KCFG_OVERRIDE"):
    import json as _json

    KCFG.update(_json.loads(_os.environ["KCFG_OVERRIDE"]))

# ---- static layout ----------------------------------------------------
_ORDER = list(range(N_SLOTS - 1, -1, -1))   # largest slot first
_GROUPS = [_ORDER[g * GROUP : (g + 1) * GROUP] for g in range(N_GROUPS)]


def _build_units():
    """Per slot: list of (row_start, unit_rows). PE base partitions allow
    pb in {0,64} for 64-row operands, so remainder units are 64 or 128."""
    units = {}
    for j in range(N_SLOTS):
        NJ = 8 * (j + 1)
        f = NJ // 128
        rem = NJ - 128 * f
        us = [(128 * c, 128) for c in range(f)]
        if rem:
            if KCFG["pack"] == "pad64" and rem <= 64:
                us.append((128 * f, 64))
            else:
                us.append((128 * f, 128))
        units[j] = us
    return units


def _build_strips():
    """Stack units into 128-row strips: [(j, row_start, rows, pb)].

    Full units get their own strip. 64-row units pair with the
    neighbouring such slot (within a 16-slot band, descending j) at
    partition offsets 0/64; widths differ by <=56 cols so mismatch
    waste stays tiny. Strips are emitted largest slot first.
    """
    units = _build_units()
    halves = {}
    for j in range(N_SLOTS):
        for rs, rows in units[j]:
            if rows == 64:
                halves[j] = rs
    half_groups = {}
    for t in range(4):
        hs = [j for j in range(16 * t + 15, 16 * t - 1, -1) if j in halves]
        for a in range(0, len(hs) - 1, 2):
            half_groups[hs[a]] = hs[a : a + 2]
        if len(hs) % 2:
            half_groups[hs[-1]] = [hs[-1]]
    strips = []
    done = set()
    for j in _ORDER:
        for rs, rows in units[j]:
            if rows == 128:
                strips.append([(j, rs, 128, 0)])
            else:
                if j in done:
                    continue
                grp = half_groups.get(j)
                if grp is None:
                    continue
                mem = []
                pb = 0
                for p in grp:
                    mem.append((p, halves[p], 64, pb))
                    pb += 64
                strips.append(mem)
                done.update(grp)
    return strips


def _wcapv(w):
    # small first windows so the first matmuls start early
    return (512, 1024, 2048)[w] if w < 3 else 3072


D_WIN_CAP = 512  # D window cols (16 strips each)

# chunk placement: j -> list of (vwin, vcb, dwin, dcb, pb, rows, row_start)
_SLOT_CHUNKS = {j: [] for j in range(N_SLOTS)}
_VWIN_W = []
_DWIN_W = []
_STRIP_DWIN = []   # strip idx -> d window idx
_cur_vw, _cur_vc = 0, 0
_cur_dw, _cur_dc = 0, 0
_STRIPS = _build_strips()
_VW_NEED_D = {}    # v window -> max d window required
for _si, _members in enumerate(_STRIPS):
    _sw = max(8 * (_j + 1) for _j, _, _, _ in _members)
    if _cur_vc + _sw > _wcapv(_cur_vw):
        _VWIN_W.append(_cur_vc)
        _cur_vw += 1
        _cur_vc = 0
    if _cur_dc + 32 > D_WIN_CAP:
        _DWIN_W.append(_cur_dc)
        _cur_dw += 1
        _cur_dc = 0
    for _j, _rs, _rows, _pb in _members:
        _SLOT_CHUNKS[_j].append(
            (_cur_vw, _cur_vc, _cur_dw, _cur_dc, _pb, _rows, _rs)
        )
    _VW_NEED_D[_cur_vw] = _cur_dw
    _cur_vc += _sw
    _cur_dc += 32
_VWIN_W.append(_cur_vc)
_DWIN_W.append(_cur_dc)
N_VWINS = len(_VWIN_W)
N_DWINS = len(_DWIN_W)
for _j in range(N_SLOTS):
    _SLOT_CHUNKS[_j].sort(key=lambda c: c[6])

_VWIN_OFF = []
_boff = 0
for _w in range(N_VWINS):
    _VWIN_OFF.append(_boff)
    _boff += 128 * _VWIN_W[_w]
VBLOB_ELEMS = _boff
_DWIN_OFF = []
_boff = 0
for _w in range(N_DWINS):
    _DWIN_OFF.append(_boff)
    _boff += 128 * _DWIN_W[_w]
DBLOB_ELEMS = _boff

# psum group column layout (bank-aligned, no matmul straddles a bank).
# The psum->stage copies compact the bank-alignment gaps away, so the
# stage/store/output layout uses gap-free "compact" columns.
_BANK = 512
_GROUP_COLS = []    # g -> [(j, psum_col)]
_GROUP_W = []       # g -> psum tile width (with gaps)
_GROUP_RANGES = []  # g -> [(psum_off, compact_off, width)] copy ranges
_GROUP_CW = []      # g -> compact width
for _slots in _GROUPS:
    _col = 0
    _ccol = 0
    _cols = []
    _ranges = []
    for _j in _slots:
        _NJ = 8 * (_j + 1)
        if _col // _BANK != (_col + _NJ - 1) // _BANK:
            _col = ((_col + _BANK - 1) // _BANK) * _BANK
        if _ranges and _ranges[-1][0] + _ranges[-1][2] == _col:
            _ranges[-1] = (_ranges[-1][0], _ranges[-1][1], _ranges[-1][2] + _NJ)
        else:
            _ranges.append((_col, _ccol, _NJ))
        _cols.append((_j, _col))
        _col += _NJ
        _ccol += _NJ
    _GROUP_COLS.append(_cols)
    _GROUP_W.append(_col)
    _GROUP_RANGES.append(_ranges)
    _GROUP_CW.append(_ccol)

# store groups: consecutive psum groups share one stage tile/store
# (each store pays a ~1-2us HBM completion round trip on its queue;
# fewer, larger stores stop the back half from pacing on them)
_SGN = max(1, KCFG["store_group"])
_SGROUPS = [
    list(range(g, min(g + _SGN, N_GROUPS))) for g in range(0, N_GROUPS, _SGN)
]
_SG_OF_GROUP = {}   # g -> (sg index, col offset of g inside sg stage)
_SG_CW = []
_SG_OFF = []
_goff = 0
for _si, _gl in enumerate(_SGROUPS):
    _off = 0
    for _g in _gl:
        _SG_OF_GROUP[_g] = (_si, _off)
        _off += _GROUP_CW[_g]
    _SG_CW.append(_off)
    _SG_OFF.append(_goff)
    _goff += B * _off
OUT_ELEMS = _goff

_SLOT_OUT = {}      # j -> (sg, compact col within sg stage)
for _g in range(N_GROUPS):
    _si, _goffc = _SG_OF_GROUP[_g]
    _ccol = 0
    for _j, _col in _GROUP_COLS[_g]:
        _SLOT_OUT[_j] = (_si, _goffc + _ccol)
        _ccol += 8 * (_j + 1)

# load issue order: D windows interleave just before first use
_LOAD_ORDER = []
_d_done = -1
for _w in range(N_VWINS):
    while _d_done < _VW_NEED_D.get(_w, -1):
        _d_done += 1
        _LOAD_ORDER.append(("d", _d_done))
    _LOAD_ORDER.append(("v", _w))
while _d_done < N_DWINS - 1:
    _d_done += 1
    _LOAD_ORDER.append(("d", _d_done))

_compiled_nc = None


def _dt(name):
    return {
        "f32": mybir.dt.float32,
        "f32r": mybir.dt.float32r,
        "bf16": mybir.dt.bfloat16,
        "f16": mybir.dt.float16,
        "e3m4": mybir.dt.float8e3,
        "e4m3": mybir.dt.float8e4,
    }[name]


def _build_program():
    global _compiled_nc
    if _compiled_nc is not None:
        return _compiled_nc

    from contextlib import ExitStack

    nc = bacc.Bacc("TRN2", target_bir_lowering=False, debug=False)
    f32 = mybir.dt.float32
    v_dt = _dt(KCFG["v_dt"])
    d_dt = _dt(KCFG["d_dt"])
    out_dt = _dt(KCFG["out_dt"])
    vblob = nc.dram_tensor("vblob", [VBLOB_ELEMS], v_dt, kind="ExternalInput").ap()
    dblob = nc.dram_tensor("dblob", [DBLOB_ELEMS], d_dt, kind="ExternalInput").ap()
    outb = nc.dram_tensor("outblob", [OUT_ELEMS], out_dt, kind="ExternalOutput").ap()

    store_engs = [getattr(nc, e) for e in KCFG["store_engines"]]
    copy_engs = [getattr(nc, e) for e in KCFG["copy_engines"]]
    v_eng = getattr(nc, KCFG["v_queue"])
    d_eng = getattr(nc, KCFG["d_queue"])

    def _load_eng(kind, idx):
        if not KCFG["head_hwdge"]:
            return v_eng if kind == "v" else d_eng
        if kind == "d":
            return nc.scalar if idx == 0 else d_eng
        if idx == 0:
            return nc.scalar
        if idx <= KCFG["v_head_sync"]:
            return nc.sync
        return v_eng

    with tile.TileContext(nc) as tc, ExitStack() as ctx:
        win_pool = ctx.enter_context(tc.tile_pool(name="win", bufs=1))
        stage_pool = ctx.enter_context(
            tc.tile_pool(name="stage", bufs=KCFG["stage_bufs"])
        )
        psum_pool = ctx.enter_context(
            tc.tile_pool(name="psum", bufs=KCFG["psum_bufs"], space="PSUM")
        )

        vwin_tiles = [None] * N_VWINS
        dwin_tiles = [None] * N_DWINS
        for kind, w in _LOAD_ORDER:
            if kind == "v":
                wf = _VWIN_W[w]
                t = win_pool.tile([128, wf], v_dt, name=f"vw{w}", tag=f"vw{w}")
                src = vblob[_VWIN_OFF[w] : _VWIN_OFF[w] + 128 * wf].rearrange(
                    "(p f) -> p f", p=128, f=wf
                )
                _load_eng("v", w).dma_start(t[:], src)
                vwin_tiles[w] = t
            else:
                wf = _DWIN_W[w]
                t = win_pool.tile([128, wf], d_dt, name=f"dw{w}", tag=f"dw{w}")
                src = dblob[_DWIN_OFF[w] : _DWIN_OFF[w] + 128 * wf].rearrange(
                    "(p f) -> p f", p=128, f=wf
                )
                _load_eng("d", w).dma_start(t[:], src)
                dwin_tiles[w] = t

        def _copy(eng, dst_ap, src_ap):
            if eng is nc.scalar:
                eng.copy(dst_ap, src_ap)
            else:
                eng.tensor_copy(dst_ap, src_ap)

        for si, gl in enumerate(_SGROUPS):
            scw = _SG_CW[si]
            stage_t = stage_pool.tile(
                [B, scw], out_dt, name=f"st{si}", tag="stage"
            )
            for g in gl:
                slots = _GROUPS[g]
                gw = _GROUP_W[g]
                gcol = _SG_OF_GROUP[g][1]
                psum_t = psum_pool.tile([B, gw], f32, name=f"psum{g}", tag="psum")
                for j, col in _GROUP_COLS[g]:
                    NJ = 8 * (j + 1)
                    chs = _SLOT_CHUNKS[j]
                    for c, (vw, vcb, dw, dcb, pb, rows, _rs) in enumerate(chs):
                        vt = vwin_tiles[vw]
                        dt_ = dwin_tiles[dw]
                        nc.tensor.matmul(
                            psum_t[:, col : col + NJ],
                            dt_[pb : pb + rows, dcb : dcb + DCOL],
                            vt[pb : pb + rows, vcb : vcb + NJ],
                            start=(c == 0),
                            stop=(c == len(chs) - 1),
                            tile_position=None if pb == 0 else (pb, 0),
                        )
                ceng = copy_engs[g % len(copy_engs)]
                for po, co, wdt in _GROUP_RANGES[g]:
                    _copy(
                        ceng,
                        stage_t[:, gcol + co : gcol + co + wdt],
                        psum_t[:, po : po + wdt],
                    )
            dst = outb[_SG_OFF[si] : _SG_OFF[si] + B * scw].rearrange(
                "(p w) -> p w", p=B, w=scw
            )
            store_engs[si % len(store_engs)].dma_start(dst, stage_t[:])

    nc.compile()
    _compiled_nc = nc
    return nc


def _np_dt(name):
    import ml_dtypes

    return {
        "bf16": ml_dtypes.bfloat16,
        "f16": np.float16,
        "f32": np.float32,
        "e3m4": ml_dtypes.float8_e3m4,
        "e4m3": ml_dtypes.float8_e4m3,
    }[name]


def _pack_core(k, x, W):
    vnp = _np_dt(KCFG["v_dt"])
    dnp = _np_dt(KCFG["d_dt"])
    vblob = np.zeros(VBLOB_ELEMS, vnp)
    dblob = np.zeros(DBLOB_ELEMS, dnp)
    for j in range(N_SLOTS):
        i = N_CORES * j + k
        ni = i + 1
        NJ = 8 * (j + 1)
        r = np.arange(ni)
        Dt = np.zeros((NJ, DCOL), np.float32)
        Dt[:ni] = x[:, r, i - r].T                       # D^T[r, b]
        V = np.zeros((NJ, NJ), np.float32)
        V[:ni, :ni] = np.clip(W[i, :ni, :ni].T * W_SCALE, -F8_MAX, F8_MAX)
        for vw, vcb, dw, dcb, pb, rows, rs in _SLOT_CHUNKS[j]:
            vimg = vblob[_VWIN_OFF[vw] : _VWIN_OFF[vw] + 128 * _VWIN_W[vw]]
            vimg = vimg.reshape(128, _VWIN_W[vw])
            vrl = V[rs : rs + rows]          # may be shorter than rows
            vimg[pb : pb + vrl.shape[0], vcb : vcb + NJ] = vrl.astype(vnp)
            dimg = dblob[_DWIN_OFF[dw] : _DWIN_OFF[dw] + 128 * _DWIN_W[dw]]
            dimg = dimg.reshape(128, _DWIN_W[dw])
            drl = Dt[rs : rs + rows]
            dimg[pb : pb + drl.shape[0], dcb : dcb + DCOL] = drl.astype(dnp)
    return vblob, dblob


def kernel(x, W, b):
    x = np.asarray(x, np.float32)
    W = np.asarray(W, np.float32)
    b = np.asarray(b, np.float32)

    nc = _build_program()
    in_maps = []
    for k in range(N_CORES):
        vb, db = _pack_core(k, x, W)
        in_maps.append({"vblob": vb, "dblob": db})
    res = run_bass_kernel_spmd(nc, in_maps, list(range(N_CORES)))

    y = x.copy()
    inv = 1.0 / W_SCALE
    for k in range(N_CORES):
        ob = res.results[k]["outblob"]
        for j in range(N_SLOTS):
            i = N_CORES * j + k
            ni = i + 1
            si, col = _SLOT_OUT[j]
            scw = _SG_CW[si]
            blk = np.asarray(
                ob[_SG_OFF[si] : _SG_OFF[si] + B * scw], np.float32
            ).reshape(B, scw)
            q = np.arange(ni)
            y[:, q, i - q] = blk[:, col : col + ni] * inv + b[i, :ni][None]
    return y


def emulate(x, W, b):
    """Exact host emulation of the device numeric path (for testing)."""
    x = np.asarray(x, np.float32)
    W = np.asarray(W, np.float32)
    b = np.asarray(b, np.float32)
    out_np = _np_dt(KCFG["out_dt"])
    y = x.copy()
    inv = 1.0 / W_SCALE
    for k in range(N_CORES):
        vb, db = _pack_core(k, x, W)
        for j in range(N_SLOTS):
            i = N_CORES * j + k
            ni = i + 1
            NJ = 8 * (j + 1)
            acc = np.zeros((B, NJ), np.float32)
            for vw, vcb, dw, dcb, pb, rows, _rs in _SLOT_CHUNKS[j]:
                vimg = vb[_VWIN_OFF[vw] : _VWIN_OFF[vw] + 128 * _VWIN_W[vw]]
                vimg = vimg.reshape(128, _VWIN_W[vw])
                dimg = db[_DWIN_OFF[dw] : _DWIN_OFF[dw] + 128 * _DWIN_W[dw]]
                dimg = dimg.reshape(128, _DWIN_W[dw])
                vch = vimg[pb : pb + rows, vcb : vcb + NJ].astype(np.float32)
                dch = dimg[pb : pb + rows, dcb : dcb + DCOL].astype(np.float32)
                acc += dch.T @ vch
            stg = acc.astype(out_np).astype(np.float32)
            q = np.arange(ni)
            y[:, q, i - q] = stg[:, :ni] * inv + b[i, :ni][None]
    return y


if __name__ == "__main__":
    vb = VBLOB_ELEMS * {"e3m4": 1, "e4m3": 1, "bf16": 2, "f32": 4}[KCFG["v_dt"]]
    db = DBLOB_ELEMS * {"bf16": 2, "f32": 4}[KCFG["d_dt"]]
    ob = OUT_ELEMS * {"f16": 2, "bf16": 2, "f32": 4}[KCFG["out_dt"]]
    print(f"V windows: {N_VWINS} ({_VWIN_W}), bytes {vb}")
    print(f"D windows: {N_DWINS} ({_DWIN_W}), bytes {db}")
    print(f"out bytes {ob}; total DMA {(vb + db + ob) / 1e6:.2f} MB")
    print(f"PE cols {sum(NJ for j in range(N_SLOTS) for NJ in [8 * (j + 1)] for _ in _SLOT_CHUNKS[j])}")


# revision 15
# speedup vs baseline: 1.3071x; 1.0468x over previous
"""Trainium2 Bass kernel for nn_DiagonalTraining (ragged per-anti-diagonal linear).

Math (reference): for each batch image x[b] (SxS) and each anti-diagonal
i (elements x[b, r, i-r], r=0..i), apply a per-diagonal linear layer:
  out[b,i,q] = sum_{r<=i} x[b,r,i-r] * W[i,q,r] + bias[i,q]   (q <= i)
and scatter back: y[b,q,i-q] = out[b,i,q]; positions with r+c >= S keep x.

Distribution: diagonal i -> core i%8, slot j=i//8 (64 slots per core,
balanced by construction). Host packs, per (core, slot), two operand
planes whose rows are the contraction axis r (zero-padded to the
core-independent NJ=8*(j+1) so the SPMD program is identical on all
cores):
  D^T[r,b] = x[b,r,i-r]        (bf16, 32 cols)
  V[r,q]   = 32 * W[i,q,r]     (fp8 e3m4, NJ cols; x32 keeps the
                                0.05-scaled weights in e3m4's normal
                                range [0.25,15.5]; descaled on host)
Mixed-precision halves the dominant HBM traffic (V is ~85% of bytes)
while keeping max rel-err ~1.3e-2 (< 2e-2 gate): the fp8 noise lives
only on W; D stays bf16. The per-diagonal bias is added on the host
while scattering results back (elementwise, ~0.05% of the FLOPs).

Device: slots are split into row-chunks (contraction tiles): full
128-row units plus a remainder unit padded to 64 rows when the live
remainder fits (else 128) -- "pad64". 64-row units of neighbouring
slots pair into one 128-row strip at partition offsets {0,64}. Each
strip occupies a V-window column range (width NJ) and a D-window range
(width 32) at the same partition offsets. All windows are SBUF
resident; loads are issued up front in consumption order, D windows
interleaved just before the first V window that needs them. Matmuls
(lhsT = D^T chunk [rows,32] bf16, rhs = V chunk [rows,NJ] fp8)
accumulate psum[32, NJ] per slot inside a bank-packed 4-slot group
psum tile; per group one copy (alternating vector/scalar engines)
downcasts psum->fp16 stage (fp16 keeps output quantization noise at
~5e-4, preserving the fp8 error budget) and a store DMA writes it out,
overlapping remaining loads.
"""

import sys

for _p in ("/opt/trn_rl_repo", "/opt/pypackages"):
    if _p not in sys.path:
        sys.path.append(_p)

import numpy as np

import concourse.bass as bass  # noqa: F401
import concourse.tile as tile
from concourse import bacc, mybir
from concourse.bass_utils import run_bass_kernel_spmd

B = 32          # batch
S = 512         # seq len / number of diagonals
N_CORES = 8
N_SLOTS = S // N_CORES  # 64 slots per core
DCOL = B        # width of the D^T block (matmul M axis)
GROUP = 2       # slots per psum group
N_GROUPS = N_SLOTS // GROUP
W_SCALE = 32.0  # host-side V prescale for e3m4 range
F8_MAX = 15.5   # e3m4 max finite

KCFG = {
    "v_dt": "e3m4",      # V blob dtype
    "d_dt": "bf16",      # D blob dtype
    "out_dt": "f16",     # output blob dtype
    "psum_bufs": 4,
    "stage_bufs": 4,
    "copy_engines": ("vector", "scalar"),
    # alternate store queues: one HWDGE ring serializes the ~1-2us HBM
    # completion receipt per store, pacing the whole back half
    "store_engines": ("sync", "scalar"),
    "v_queue": "gpsimd",
    "d_queue": "gpsimd",
    "pack": "pad128",    # pad64 tile_position=(64,0) crashes the PE; pad128 only
    # NOTE (measured): every engine has a ~7-9us init preamble (TENSOR_LOAD,
    # ACT tables, sem chains) before user work issues — HWDGE "head" loads
    # and PE-warmup matmuls cannot beat the SWDGE queue to first byte and
    # only delay the stream; keep all loads on the gpsimd SWDGE queue.
    "head_hwdge": False,
    "v_head_sync": 0,
    "warm_mm": 0,
    # groups per store-group: merged stages cut the store count (and the
    # per-store completion round trips) in half
    "store_group": 4,
}

# test-only bisection hook (harness never sets this)
import os as _os  # noqa: E402

if _os.environ.get("KCFG_OVERRIDE"):
    import json as _json

    KCFG.update(_json.loads(_os.environ["KCFG_OVERRIDE"]))

# ---- static layout ----------------------------------------------------
_ORDER = list(range(N_SLOTS - 1, -1, -1))   # largest slot first
_GROUPS = [_ORDER[g * GROUP : (g + 1) * GROUP] for g in range(N_GROUPS)]


def _build_units():
    """Per slot: list of (row_start, unit_rows). PE base partitions allow
    pb in {0,64} for 64-row operands, so remainder units are 64 or 128."""
    units = {}
    for j in range(N_SLOTS):
        NJ = 8 * (j + 1)
        f = NJ // 128
        rem = NJ - 128 * f
        us = [(128 * c, 128) for c in range(f)]
        if rem:
            if KCFG["pack"] == "pad64" and rem <= 64:
                us.append((128 * f, 64))
            else:
                us.append((128 * f, 128))
        units[j] = us
    return units


def _build_strips():
    """Stack units into 128-row strips: [(j, row_start, rows, pb)].

    Full units get their own strip. 64-row units pair with the
    neighbouring such slot (within a 16-slot band, descending j) at
    partition offsets 0/64; widths differ by <=56 cols so mismatch
    waste stays tiny. Strips are emitted largest slot first.
    """
    units = _build_units()
    halves = {}
    for j in range(N_SLOTS):
        for rs, rows in units[j]:
            if rows == 64:
                halves[j] = rs
    half_groups = {}
    for t in range(4):
        hs = [j for j in range(16 * t + 15, 16 * t - 1, -1) if j in halves]
        for a in range(0, len(hs) - 1, 2):
            half_groups[hs[a]] = hs[a : a + 2]
        if len(hs) % 2:
            half_groups[hs[-1]] = [hs[-1]]
    strips = []
    done = set()
    for j in _ORDER:
        for rs, rows in units[j]:
            if rows == 128:
                strips.append([(j, rs, 128, 0)])
            else:
                if j in done:
                    continue
                grp = half_groups.get(j)
                if grp is None:
                    continue
                mem = []
                pb = 0
                for p in grp:
                    mem.append((p, halves[p], 64, pb))
                    pb += 64
                strips.append(mem)
                done.update(grp)
    return strips


def _wcapv(w):
    # small first windows so the first matmuls start early
    return (512, 1024, 2048)[w] if w < 3 else 3072


D_WIN_CAP = 512  # D window cols (16 strips each)

# chunk placement: j -> list of (vwin, vcb, dwin, dcb, pb, rows, row_start)
_SLOT_CHUNKS = {j: [] for j in range(N_SLOTS)}
_VWIN_W = []
_DWIN_W = []
_STRIP_DWIN = []   # strip idx -> d window idx
_cur_vw, _cur_vc = 0, 0
_cur_dw, _cur_dc = 0, 0
_STRIPS = _build_strips()
_VW_NEED_D = {}    # v window -> max d window required
for _si, _members in enumerate(_STRIPS):
    _sw = max(8 * (_j + 1) for _j, _, _, _ in _members)
    if _cur_vc + _sw > _wcapv(_cur_vw):
        _VWIN_W.append(_cur_vc)
        _cur_vw += 1
        _cur_vc = 0
    if _cur_dc + 32 > D_WIN_CAP:
        _DWIN_W.append(_cur_dc)
        _cur_dw += 1
        _cur_dc = 0
    for _j, _rs, _rows, _pb in _members:
        _SLOT_CHUNKS[_j].append(
            (_cur_vw, _cur_vc, _cur_dw, _cur_dc, _pb, _rows, _rs)
        )
    _VW_NEED_D[_cur_vw] = _cur_dw
    _cur_vc += _sw
    _cur_dc += 32
_VWIN_W.append(_cur_vc)
_DWIN_W.append(_cur_dc)
N_VWINS = len(_VWIN_W)
N_DWINS = len(_DWIN_W)
for _j in range(N_SLOTS):
    _SLOT_CHUNKS[_j].sort(key=lambda c: c[6])

_VWIN_OFF = []
_boff = 0
for _w in range(N_VWINS):
    _VWIN_OFF.append(_boff)
    _boff += 128 * _VWIN_W[_w]
VBLOB_ELEMS = _boff
_DWIN_OFF = []
_boff = 0
for _w in range(N_DWINS):
    _DWIN_OFF.append(_boff)
    _boff += 128 * _DWIN_W[_w]
DBLOB_ELEMS = _boff

# psum group column layout (bank-aligned, no matmul straddles a bank).
# The psum->stage copies compact the bank-alignment gaps away, so the
# stage/store/output layout uses gap-free "compact" columns.
_BANK = 512
_GROUP_COLS = []    # g -> [(j, psum_col)]
_GROUP_W = []       # g -> psum tile width (with gaps)
_GROUP_RANGES = []  # g -> [(psum_off, compact_off, width)] copy ranges
_GROUP_CW = []      # g -> compact width
for _slots in _GROUPS:
    _col = 0
    _ccol = 0
    _cols = []
    _ranges = []
    for _j in _slots:
        _NJ = 8 * (_j + 1)
        if _col // _BANK != (_col + _NJ - 1) // _BANK:
            _col = ((_col + _BANK - 1) // _BANK) * _BANK
        if _ranges and _ranges[-1][0] + _ranges[-1][2] == _col:
            _ranges[-1] = (_ranges[-1][0], _ranges[-1][1], _ranges[-1][2] + _NJ)
        else:
            _ranges.append((_col, _ccol, _NJ))
        _cols.append((_j, _col))
        _col += _NJ
        _ccol += _NJ
    _GROUP_COLS.append(_cols)
    _GROUP_W.append(_col)
    _GROUP_RANGES.append(_ranges)
    _GROUP_CW.append(_ccol)

# store groups: consecutive psum groups share one stage tile/store
# (each store pays a ~1-2us HBM completion round trip on its queue;
# fewer, larger stores stop the back half from pacing on them)
_SGN = max(1, KCFG["store_group"])
_SGROUPS = [
    list(range(g, min(g + _SGN, N_GROUPS))) for g in range(0, N_GROUPS, _SGN)
]
_SG_OF_GROUP = {}   # g -> (sg index, col offset of g inside sg stage)
_SG_CW = []
_SG_OFF = []
_goff = 0
for _si, _gl in enumerate(_SGROUPS):
    _off = 0
    for _g in _gl:
        _SG_OF_GROUP[_g] = (_si, _off)
        _off += _GROUP_CW[_g]
    _SG_CW.append(_off)
    _SG_OFF.append(_goff)
    _goff += B * _off
OUT_ELEMS = _goff

_SLOT_OUT = {}      # j -> (sg, compact col within sg stage)
for _g in range(N_GROUPS):
    _si, _goffc = _SG_OF_GROUP[_g]
    _ccol = 0
    for _j, _col in _GROUP_COLS[_g]:
        _SLOT_OUT[_j] = (_si, _goffc + _ccol)
        _ccol += 8 * (_j + 1)

# load issue order: D windows interleave just before first use
_LOAD_ORDER = []
_d_done = -1
for _w in range(N_VWINS):
    while _d_done < _VW_NEED_D.get(_w, -1):
        _d_done += 1
        _LOAD_ORDER.append(("d", _d_done))
    _LOAD_ORDER.append(("v", _w))
while _d_done < N_DWINS - 1:
    _d_done += 1
    _LOAD_ORDER.append(("d", _d_done))

_compiled_nc = None


def _dt(name):
    return {
        "f32": mybir.dt.float32,
        "f32r": mybir.dt.float32r,
        "bf16": mybir.dt.bfloat16,
        "f16": mybir.dt.float16,
        "e3m4": mybir.dt.float8e3,
        "e4m3": mybir.dt.float8e4,
    }[name]


def _build_program():
    global _compiled_nc
    if _compiled_nc is not None:
        return _compiled_nc

    from contextlib import ExitStack

    nc = bacc.Bacc("TRN2", target_bir_lowering=False, debug=False)
    f32 = mybir.dt.float32
    v_dt = _dt(KCFG["v_dt"])
    d_dt = _dt(KCFG["d_dt"])
    out_dt = _dt(KCFG["out_dt"])
    vblob = nc.dram_tensor("vblob", [VBLOB_ELEMS], v_dt, kind="ExternalInput").ap()
    dblob = nc.dram_tensor("dblob", [DBLOB_ELEMS], d_dt, kind="ExternalInput").ap()
    outb = nc.dram_tensor("outblob", [OUT_ELEMS], out_dt, kind="ExternalOutput").ap()

    store_engs = [getattr(nc, e) for e in KCFG["store_engines"]]
    copy_engs = [getattr(nc, e) for e in KCFG["copy_engines"]]
    v_eng = getattr(nc, KCFG["v_queue"])
    d_eng = getattr(nc, KCFG["d_queue"])

    def _load_eng(kind, idx):
        if not KCFG["head_hwdge"]:
            return v_eng if kind == "v" else d_eng
        if kind == "d":
            return nc.scalar if idx == 0 else d_eng
        if idx == 0:
            return nc.scalar
        if idx <= KCFG["v_head_sync"]:
            return nc.sync
        return v_eng

    with tile.TileContext(nc) as tc, ExitStack() as ctx:
        win_pool = ctx.enter_context(tc.tile_pool(name="win", bufs=1))
        stage_pool = ctx.enter_context(
            tc.tile_pool(name="stage", bufs=KCFG["stage_bufs"])
        )
        psum_pool = ctx.enter_context(
            tc.tile_pool(name="psum", bufs=KCFG["psum_bufs"], space="PSUM")
        )

        vwin_tiles = [None] * N_VWINS
        dwin_tiles = [None] * N_DWINS
        for kind, w in _LOAD_ORDER:
            if kind == "v":
                wf = _VWIN_W[w]
                t = win_pool.tile([128, wf], v_dt, name=f"vw{w}", tag=f"vw{w}")
                src = vblob[_VWIN_OFF[w] : _VWIN_OFF[w] + 128 * wf].rearrange(
                    "(p f) -> p f", p=128, f=wf
                )
                _load_eng("v", w).dma_start(t[:], src)
                vwin_tiles[w] = t
            else:
                wf = _DWIN_W[w]
                t = win_pool.tile([128, wf], d_dt, name=f"dw{w}", tag=f"dw{w}")
                src = dblob[_DWIN_OFF[w] : _DWIN_OFF[w] + 128 * wf].rearrange(
                    "(p f) -> p f", p=128, f=wf
                )
                _load_eng("d", w).dma_start(t[:], src)
                dwin_tiles[w] = t

        def _copy(eng, dst_ap, src_ap):
            if eng is nc.scalar:
                eng.copy(dst_ap, src_ap)
            else:
                eng.tensor_copy(dst_ap, src_ap)

        for si, gl in enumerate(_SGROUPS):
            scw = _SG_CW[si]
            stage_t = stage_pool.tile(
                [B, scw], out_dt, name=f"st{si}", tag="stage"
            )
            for g in gl:
                slots = _GROUPS[g]
                gw = _GROUP_W[g]
                gcol = _SG_OF_GROUP[g][1]
                # allocate whole PSUM banks so the pool can't place the
                # tile mid-bank (the column layout assumes bank alignment)
                gwa = ((gw + _BANK - 1) // _BANK) * _BANK
                psum_t = psum_pool.tile([B, gwa], f32, name=f"psum{g}", tag="psum")
                for j, col in _GROUP_COLS[g]:
                    NJ = 8 * (j + 1)
                    chs = _SLOT_CHUNKS[j]
                    for c, (vw, vcb, dw, dcb, pb, rows, _rs) in enumerate(chs):
                        vt = vwin_tiles[vw]
                        dt_ = dwin_tiles[dw]
                        nc.tensor.matmul(
                            psum_t[:, col : col + NJ],
                            dt_[pb : pb + rows, dcb : dcb + DCOL],
                            vt[pb : pb + rows, vcb : vcb + NJ],
                            start=(c == 0),
                            stop=(c == len(chs) - 1),
                            tile_position=None if pb == 0 else (pb, 0),
                        )
                ceng = copy_engs[g % len(copy_engs)]
                for po, co, wdt in _GROUP_RANGES[g]:
                    _copy(
                        ceng,
                        stage_t[:, gcol + co : gcol + co + wdt],
                        psum_t[:, po : po + wdt],
                    )
            dst = outb[_SG_OFF[si] : _SG_OFF[si] + B * scw].rearrange(
                "(p w) -> p w", p=B, w=scw
            )
            store_engs[si % len(store_engs)].dma_start(dst, stage_t[:])

    nc.compile()
    _compiled_nc = nc
    return nc


def _np_dt(name):
    import ml_dtypes

    return {
        "bf16": ml_dtypes.bfloat16,
        "f16": np.float16,
        "f32": np.float32,
        "e3m4": ml_dtypes.float8_e3m4,
        "e4m3": ml_dtypes.float8_e4m3,
    }[name]


def _pack_core(k, x, W):
    vnp = _np_dt(KCFG["v_dt"])
    dnp = _np_dt(KCFG["d_dt"])
    vblob = np.zeros(VBLOB_ELEMS, vnp)
    dblob = np.zeros(DBLOB_ELEMS, dnp)
    for j in range(N_SLOTS):
        i = N_CORES * j + k
        ni = i + 1
        NJ = 8 * (j + 1)
        r = np.arange(ni)
        Dt = np.zeros((NJ, DCOL), np.float32)
        Dt[:ni] = x[:, r, i - r].T                       # D^T[r, b]
        V = np.zeros((NJ, NJ), np.float32)
        V[:ni, :ni] = np.clip(W[i, :ni, :ni].T * W_SCALE, -F8_MAX, F8_MAX)
        for vw, vcb, dw, dcb, pb, rows, rs in _SLOT_CHUNKS[j]:
            vimg = vblob[_VWIN_OFF[vw] : _VWIN_OFF[vw] + 128 * _VWIN_W[vw]]
            vimg = vimg.reshape(128, _VWIN_W[vw])
            vrl = V[rs : rs + rows]          # may be shorter than rows
            vimg[pb : pb + vrl.shape[0], vcb : vcb + NJ] = vrl.astype(vnp)
            dimg = dblob[_DWIN_OFF[dw] : _DWIN_OFF[dw] + 128 * _DWIN_W[dw]]
            dimg = dimg.reshape(128, _DWIN_W[dw])
            drl = Dt[rs : rs + rows]
            dimg[pb : pb + drl.shape[0], dcb : dcb + DCOL] = drl.astype(dnp)
    return vblob, dblob


def kernel(x, W, b):
    x = np.asarray(x, np.float32)
    W = np.asarray(W, np.float32)
    b = np.asarray(b, np.float32)

    nc = _build_program()
    in_maps = []
    for k in range(N_CORES):
        vb, db = _pack_core(k, x, W)
        in_maps.append({"vblob": vb, "dblob": db})
    res = run_bass_kernel_spmd(nc, in_maps, list(range(N_CORES)))

    y = x.copy()
    inv = 1.0 / W_SCALE
    for k in range(N_CORES):
        ob = res.results[k]["outblob"]
        for j in range(N_SLOTS):
            i = N_CORES * j + k
            ni = i + 1
            si, col = _SLOT_OUT[j]
            scw = _SG_CW[si]
            blk = np.asarray(
                ob[_SG_OFF[si] : _SG_OFF[si] + B * scw], np.float32
            ).reshape(B, scw)
            q = np.arange(ni)
            y[:, q, i - q] = blk[:, col : col + ni] * inv + b[i, :ni][None]
    return y


def emulate(x, W, b):
    """Exact host emulation of the device numeric path (for testing)."""
    x = np.asarray(x, np.float32)
    W = np.asarray(W, np.float32)
    b = np.asarray(b, np.float32)
    out_np = _np_dt(KCFG["out_dt"])
    y = x.copy()
    inv = 1.0 / W_SCALE
    for k in range(N_CORES):
        vb, db = _pack_core(k, x, W)
        for j in range(N_SLOTS):
            i = N_CORES * j + k
            ni = i + 1
            NJ = 8 * (j + 1)
            acc = np.zeros((B, NJ), np.float32)
            for vw, vcb, dw, dcb, pb, rows, _rs in _SLOT_CHUNKS[j]:
                vimg = vb[_VWIN_OFF[vw] : _VWIN_OFF[vw] + 128 * _VWIN_W[vw]]
                vimg = vimg.reshape(128, _VWIN_W[vw])
                dimg = db[_DWIN_OFF[dw] : _DWIN_OFF[dw] + 128 * _DWIN_W[dw]]
                dimg = dimg.reshape(128, _DWIN_W[dw])
                vch = vimg[pb : pb + rows, vcb : vcb + NJ].astype(np.float32)
                dch = dimg[pb : pb + rows, dcb : dcb + DCOL].astype(np.float32)
                acc += dch.T @ vch
            stg = acc.astype(out_np).astype(np.float32)
            q = np.arange(ni)
            y[:, q, i - q] = stg[:, :ni] * inv + b[i, :ni][None]
    return y


if __name__ == "__main__":
    vb = VBLOB_ELEMS * {"e3m4": 1, "e4m3": 1, "bf16": 2, "f32": 4}[KCFG["v_dt"]]
    db = DBLOB_ELEMS * {"bf16": 2, "f32": 4}[KCFG["d_dt"]]
    ob = OUT_ELEMS * {"f16": 2, "bf16": 2, "f32": 4}[KCFG["out_dt"]]
    print(f"V windows: {N_VWINS} ({_VWIN_W}), bytes {vb}")
    print(f"D windows: {N_DWINS} ({_DWIN_W}), bytes {db}")
    print(f"out bytes {ob}; total DMA {(vb + db + ob) / 1e6:.2f} MB")
    print(f"PE cols {sum(NJ for j in range(N_SLOTS) for NJ in [8 * (j + 1)] for _ in _SLOT_CHUNKS[j])}")


# revision 16
# speedup vs baseline: 1.3363x; 1.0223x over previous
"""Trainium2 Bass kernel for nn_DiagonalTraining (ragged per-anti-diagonal linear).

Math (reference): for each batch image x[b] (SxS) and each anti-diagonal
i (elements x[b, r, i-r], r=0..i), apply a per-diagonal linear layer:
  out[b,i,q] = sum_{r<=i} x[b,r,i-r] * W[i,q,r] + bias[i,q]   (q <= i)
and scatter back: y[b,q,i-q] = out[b,i,q]; positions with r+c >= S keep x.

Distribution: diagonal i -> core i%8, slot j=i//8 (64 slots per core,
balanced by construction). Host packs, per (core, slot), two operand
planes whose rows are the contraction axis r (zero-padded to the
core-independent NJ=8*(j+1) so the SPMD program is identical on all
cores):
  D^T[r,b] = x[b,r,i-r]        (bf16, 32 cols)
  V[r,q]   = 32 * W[i,q,r]     (fp8 e3m4, NJ cols; x32 keeps the
                                0.05-scaled weights in e3m4's normal
                                range [0.25,15.5]; descaled on host)
Mixed-precision halves the dominant HBM traffic (V is ~85% of bytes)
while keeping max rel-err ~1.3e-2 (< 2e-2 gate): the fp8 noise lives
only on W; D stays bf16. The per-diagonal bias is added on the host
while scattering results back (elementwise, ~0.05% of the FLOPs).

Device: slots are split into row-chunks (contraction tiles): full
128-row units plus a remainder unit padded to 64 rows when the live
remainder fits (else 128) -- "pad64". 64-row units of neighbouring
slots pair into one 128-row strip at partition offsets {0,64}. Each
strip occupies a V-window column range (width NJ) and a D-window range
(width 32) at the same partition offsets. All windows are SBUF
resident; loads are issued up front in consumption order, D windows
interleaved just before the first V window that needs them. Matmuls
(lhsT = D^T chunk [rows,32] bf16, rhs = V chunk [rows,NJ] fp8)
accumulate psum[32, NJ] per slot inside a bank-packed 4-slot group
psum tile; per group one copy (alternating vector/scalar engines)
downcasts psum->fp16 stage (fp16 keeps output quantization noise at
~5e-4, preserving the fp8 error budget) and a store DMA writes it out,
overlapping remaining loads.
"""

import sys

for _p in ("/opt/trn_rl_repo", "/opt/pypackages"):
    if _p not in sys.path:
        sys.path.append(_p)

import numpy as np

import concourse.bass as bass  # noqa: F401
import concourse.tile as tile
from concourse import bacc, mybir
from concourse.bass_utils import run_bass_kernel_spmd

B = 32          # batch
S = 512         # seq len / number of diagonals
N_CORES = 8
N_SLOTS = S // N_CORES  # 64 slots per core
DCOL = B        # width of the D^T block (matmul M axis)
GROUP = 1       # slots per psum group
N_GROUPS = N_SLOTS // GROUP
W_SCALE = 32.0  # host-side V prescale for e3m4 range
F8_MAX = 15.5   # e3m4 max finite

KCFG = {
    "v_dt": "e3m4",      # V blob dtype
    "d_dt": "bf16",      # D blob dtype
    "out_dt": "f16",     # output blob dtype
    "psum_bufs": 8,
    "stage_bufs": 4,
    "copy_engines": ("vector", "scalar"),
    # alternate store queues: one HWDGE ring serializes the ~1-2us HBM
    # completion receipt per store, pacing the whole back half
    "store_engines": ("sync", "scalar"),
    "v_queue": "gpsimd",
    "d_queue": "gpsimd",
    "pack": "pad128",    # pad64 tile_position=(64,0) crashes the PE; pad128 only
    # NOTE (measured): every engine has a ~7-9us init preamble (TENSOR_LOAD,
    # ACT tables, sem chains) before user work issues — HWDGE "head" loads
    # and PE-warmup matmuls cannot beat the SWDGE queue to first byte and
    # only delay the stream; keep all loads on the gpsimd SWDGE queue.
    "head_hwdge": False,
    "v_head_sync": 0,
    "warm_mm": 5,
    # groups per store-group: merged stages cut the store count (and the
    # per-store completion round trips) in half
    "store_group": 8,
}

# test-only bisection hook (harness never sets this)
import os as _os  # noqa: E402

if _os.environ.get("KCFG_OVERRIDE"):
    import json as _json

    KCFG.update(_json.loads(_os.environ["KCFG_OVERRIDE"]))

# ---- static layout ----------------------------------------------------
_ORDER = list(range(N_SLOTS - 1, -1, -1))   # largest slot first
_GROUPS = [_ORDER[g * GROUP : (g + 1) * GROUP] for g in range(N_GROUPS)]


def _build_units():
    """Per slot: list of (row_start, unit_rows). PE base partitions allow
    pb in {0,64} for 64-row operands, so remainder units are 64 or 128."""
    units = {}
    for j in range(N_SLOTS):
        NJ = 8 * (j + 1)
        f = NJ // 128
        rem = NJ - 128 * f
        us = [(128 * c, 128) for c in range(f)]
        if rem:
            if KCFG["pack"] == "pad64" and rem <= 64:
                us.append((128 * f, 64))
            else:
                us.append((128 * f, 128))
        units[j] = us
    return units


def _build_strips():
    """Stack units into 128-row strips: [(j, row_start, rows, pb)].

    Full units get their own strip. 64-row units pair with the
    neighbouring such slot (within a 16-slot band, descending j) at
    partition offsets 0/64; widths differ by <=56 cols so mismatch
    waste stays tiny. Strips are emitted largest slot first.
    """
    units = _build_units()
    halves = {}
    for j in range(N_SLOTS):
        for rs, rows in units[j]:
            if rows == 64:
                halves[j] = rs
    half_groups = {}
    for t in range(4):
        hs = [j for j in range(16 * t + 15, 16 * t - 1, -1) if j in halves]
        for a in range(0, len(hs) - 1, 2):
            half_groups[hs[a]] = hs[a : a + 2]
        if len(hs) % 2:
            half_groups[hs[-1]] = [hs[-1]]
    strips = []
    done = set()
    for j in _ORDER:
        for rs, rows in units[j]:
            if rows == 128:
                strips.append([(j, rs, 128, 0)])
            else:
                if j in done:
                    continue
                grp = half_groups.get(j)
                if grp is None:
                    continue
                mem = []
                pb = 0
                for p in grp:
                    mem.append((p, halves[p], 64, pb))
                    pb += 64
                strips.append(mem)
                done.update(grp)
    return strips


def _wcapv(w):
    # small first windows so the first matmuls start early
    return (512, 1024, 2048)[w] if w < 3 else 4096


D_WIN_CAP = 1024  # D window cols (32 strips each)

# chunk placement: j -> list of (vwin, vcb, dwin, dcb, pb, rows, row_start)
_SLOT_CHUNKS = {j: [] for j in range(N_SLOTS)}
_VWIN_W = []
_DWIN_W = []
_STRIP_DWIN = []   # strip idx -> d window idx
_cur_vw, _cur_vc = 0, 0
_cur_dw, _cur_dc = 0, 0
_STRIPS = _build_strips()
_VW_NEED_D = {}    # v window -> max d window required
for _si, _members in enumerate(_STRIPS):
    _sw = max(8 * (_j + 1) for _j, _, _, _ in _members)
    if _cur_vc + _sw > _wcapv(_cur_vw):
        _VWIN_W.append(_cur_vc)
        _cur_vw += 1
        _cur_vc = 0
    if _cur_dc + 32 > D_WIN_CAP:
        _DWIN_W.append(_cur_dc)
        _cur_dw += 1
        _cur_dc = 0
    for _j, _rs, _rows, _pb in _members:
        _SLOT_CHUNKS[_j].append(
            (_cur_vw, _cur_vc, _cur_dw, _cur_dc, _pb, _rows, _rs)
        )
    _VW_NEED_D[_cur_vw] = _cur_dw
    _cur_vc += _sw
    _cur_dc += 32
_VWIN_W.append(_cur_vc)
_DWIN_W.append(_cur_dc)
N_VWINS = len(_VWIN_W)
N_DWINS = len(_DWIN_W)
for _j in range(N_SLOTS):
    _SLOT_CHUNKS[_j].sort(key=lambda c: c[6])

_VWIN_OFF = []
_boff = 0
for _w in range(N_VWINS):
    _VWIN_OFF.append(_boff)
    _boff += 128 * _VWIN_W[_w]
VBLOB_ELEMS = _boff
_DWIN_OFF = []
_boff = 0
for _w in range(N_DWINS):
    _DWIN_OFF.append(_boff)
    _boff += 128 * _DWIN_W[_w]
DBLOB_ELEMS = _boff

# psum group column layout (bank-aligned, no matmul straddles a bank).
# The psum->stage copies compact the bank-alignment gaps away, so the
# stage/store/output layout uses gap-free "compact" columns.
_BANK = 512
_GROUP_COLS = []    # g -> [(j, psum_col)]
_GROUP_W = []       # g -> psum tile width (with gaps)
_GROUP_RANGES = []  # g -> [(psum_off, compact_off, width)] copy ranges
_GROUP_CW = []      # g -> compact width
for _slots in _GROUPS:
    _col = 0
    _ccol = 0
    _cols = []
    _ranges = []
    for _j in _slots:
        _NJ = 8 * (_j + 1)
        if _col // _BANK != (_col + _NJ - 1) // _BANK:
            _col = ((_col + _BANK - 1) // _BANK) * _BANK
        if _ranges and _ranges[-1][0] + _ranges[-1][2] == _col:
            _ranges[-1] = (_ranges[-1][0], _ranges[-1][1], _ranges[-1][2] + _NJ)
        else:
            _ranges.append((_col, _ccol, _NJ))
        _cols.append((_j, _col))
        _col += _NJ
        _ccol += _NJ
    _GROUP_COLS.append(_cols)
    _GROUP_W.append(_col)
    _GROUP_RANGES.append(_ranges)
    _GROUP_CW.append(_ccol)

# store groups: consecutive psum groups share one stage tile/store
# (each store pays a ~1-2us HBM completion round trip on its queue;
# fewer, larger stores stop the back half from pacing on them)
_SGN = max(1, KCFG["store_group"])
_SGROUPS = [
    list(range(g, min(g + _SGN, N_GROUPS))) for g in range(0, N_GROUPS, _SGN)
]
_SG_OF_GROUP = {}   # g -> (sg index, col offset of g inside sg stage)
_SG_CW = []
_SG_OFF = []
_goff = 0
for _si, _gl in enumerate(_SGROUPS):
    _off = 0
    for _g in _gl:
        _SG_OF_GROUP[_g] = (_si, _off)
        _off += _GROUP_CW[_g]
    _SG_CW.append(_off)
    _SG_OFF.append(_goff)
    _goff += B * _off
OUT_ELEMS = _goff

_SLOT_OUT = {}      # j -> (sg, compact col within sg stage)
for _g in range(N_GROUPS):
    _si, _goffc = _SG_OF_GROUP[_g]
    _ccol = 0
    for _j, _col in _GROUP_COLS[_g]:
        _SLOT_OUT[_j] = (_si, _goffc + _ccol)
        _ccol += 8 * (_j + 1)

# load issue order: D windows interleave just before first use
_LOAD_ORDER = []
_d_done = -1
for _w in range(N_VWINS):
    while _d_done < _VW_NEED_D.get(_w, -1):
        _d_done += 1
        _LOAD_ORDER.append(("d", _d_done))
    _LOAD_ORDER.append(("v", _w))
while _d_done < N_DWINS - 1:
    _d_done += 1
    _LOAD_ORDER.append(("d", _d_done))

_compiled_nc = None


def _dt(name):
    return {
        "f32": mybir.dt.float32,
        "f32r": mybir.dt.float32r,
        "bf16": mybir.dt.bfloat16,
        "f16": mybir.dt.float16,
        "e3m4": mybir.dt.float8e3,
        "e4m3": mybir.dt.float8e4,
    }[name]


def _build_program():
    global _compiled_nc
    if _compiled_nc is not None:
        return _compiled_nc

    from contextlib import ExitStack

    nc = bacc.Bacc("TRN2", target_bir_lowering=False, debug=False)
    f32 = mybir.dt.float32
    v_dt = _dt(KCFG["v_dt"])
    d_dt = _dt(KCFG["d_dt"])
    out_dt = _dt(KCFG["out_dt"])
    vblob = nc.dram_tensor("vblob", [VBLOB_ELEMS], v_dt, kind="ExternalInput").ap()
    dblob = nc.dram_tensor("dblob", [DBLOB_ELEMS], d_dt, kind="ExternalInput").ap()
    outb = nc.dram_tensor("outblob", [OUT_ELEMS], out_dt, kind="ExternalOutput").ap()

    store_engs = [getattr(nc, e) for e in KCFG["store_engines"]]
    copy_engs = [getattr(nc, e) for e in KCFG["copy_engines"]]
    v_eng = getattr(nc, KCFG["v_queue"])
    d_eng = getattr(nc, KCFG["d_queue"])

    def _load_eng(kind, idx):
        if not KCFG["head_hwdge"]:
            return v_eng if kind == "v" else d_eng
        if kind == "d":
            return nc.scalar if idx == 0 else d_eng
        if idx == 0:
            return nc.scalar
        if idx <= KCFG["v_head_sync"]:
            return nc.sync
        return v_eng

    with tile.TileContext(nc) as tc, ExitStack() as ctx:
        win_pool = ctx.enter_context(tc.tile_pool(name="win", bufs=1))
        stage_pool = ctx.enter_context(
            tc.tile_pool(name="stage", bufs=KCFG["stage_bufs"])
        )
        psum_pool = ctx.enter_context(
            tc.tile_pool(name="psum", bufs=KCFG["psum_bufs"], space="PSUM")
        )

        # PE clock warmup: the HAM gate holds the PE at 1.2GHz until
        # ~3.4us of sustained activity. The engine preamble ends ~8.3us
        # and the first data lands ~10.8us; 5 zero matmuls fill that
        # window so the real stream starts at full clock.
        if KCFG["warm_mm"]:
            warm_pool = ctx.enter_context(tc.tile_pool(name="warm", bufs=1))
            wd = warm_pool.tile([128, DCOL], d_dt, name="warm_d", tag="warm_d")
            wv = warm_pool.tile([128, 512], v_dt, name="warm_v", tag="warm_v")
            nc.vector.memset(wd[:], 0)
            nc.vector.memset(wv[:], 0)
            wp = psum_pool.tile([B, 512], f32, name="warm_p", tag="psum")
            for _ in range(KCFG["warm_mm"]):
                nc.tensor.matmul(wp[:], wd[:], wv[:], start=True, stop=True)

        vwin_tiles = [None] * N_VWINS
        dwin_tiles = [None] * N_DWINS
        for kind, w in _LOAD_ORDER:
            if kind == "v":
                wf = _VWIN_W[w]
                t = win_pool.tile([128, wf], v_dt, name=f"vw{w}", tag=f"vw{w}")
                src = vblob[_VWIN_OFF[w] : _VWIN_OFF[w] + 128 * wf].rearrange(
                    "(p f) -> p f", p=128, f=wf
                )
                _load_eng("v", w).dma_start(t[:], src)
                vwin_tiles[w] = t
            else:
                wf = _DWIN_W[w]
                t = win_pool.tile([128, wf], d_dt, name=f"dw{w}", tag=f"dw{w}")
                src = dblob[_DWIN_OFF[w] : _DWIN_OFF[w] + 128 * wf].rearrange(
                    "(p f) -> p f", p=128, f=wf
                )
                _load_eng("d", w).dma_start(t[:], src)
                dwin_tiles[w] = t

        def _copy(eng, dst_ap, src_ap):
            if eng is nc.scalar:
                eng.copy(dst_ap, src_ap)
            else:
                eng.tensor_copy(dst_ap, src_ap)

        for si, gl in enumerate(_SGROUPS):
            scw = _SG_CW[si]
            stage_t = stage_pool.tile(
                [B, scw], out_dt, name=f"st{si}", tag="stage"
            )
            for g in gl:
                slots = _GROUPS[g]
                gw = _GROUP_W[g]
                gcol = _SG_OF_GROUP[g][1]
                # allocate whole PSUM banks so the pool can't place the
                # tile mid-bank (the column layout assumes bank alignment)
                gwa = ((gw + _BANK - 1) // _BANK) * _BANK
                psum_t = psum_pool.tile([B, gwa], f32, name=f"psum{g}", tag="psum")
                for j, col in _GROUP_COLS[g]:
                    NJ = 8 * (j + 1)
                    chs = _SLOT_CHUNKS[j]
                    for c, (vw, vcb, dw, dcb, pb, rows, _rs) in enumerate(chs):
                        vt = vwin_tiles[vw]
                        dt_ = dwin_tiles[dw]
                        nc.tensor.matmul(
                            psum_t[:, col : col + NJ],
                            dt_[pb : pb + rows, dcb : dcb + DCOL],
                            vt[pb : pb + rows, vcb : vcb + NJ],
                            start=(c == 0),
                            stop=(c == len(chs) - 1),
                            tile_position=None if pb == 0 else (pb, 0),
                        )
                ceng = copy_engs[g % len(copy_engs)]
                for po, co, wdt in _GROUP_RANGES[g]:
                    _copy(
                        ceng,
                        stage_t[:, gcol + co : gcol + co + wdt],
                        psum_t[:, po : po + wdt],
                    )
            dst = outb[_SG_OFF[si] : _SG_OFF[si] + B * scw].rearrange(
                "(p w) -> p w", p=B, w=scw
            )
            store_engs[si % len(store_engs)].dma_start(dst, stage_t[:])

    nc.compile()
    _compiled_nc = nc
    return nc


def _np_dt(name):
    import ml_dtypes

    return {
        "bf16": ml_dtypes.bfloat16,
        "f16": np.float16,
        "f32": np.float32,
        "e3m4": ml_dtypes.float8_e3m4,
        "e4m3": ml_dtypes.float8_e4m3,
    }[name]


def _pack_core(k, x, W):
    vnp = _np_dt(KCFG["v_dt"])
    dnp = _np_dt(KCFG["d_dt"])
    vblob = np.zeros(VBLOB_ELEMS, vnp)
    dblob = np.zeros(DBLOB_ELEMS, dnp)
    for j in range(N_SLOTS):
        i = N_CORES * j + k
        ni = i + 1
        NJ = 8 * (j + 1)
        r = np.arange(ni)
        Dt = np.zeros((NJ, DCOL), np.float32)
        Dt[:ni] = x[:, r, i - r].T                       # D^T[r, b]
        V = np.zeros((NJ, NJ), np.float32)
        V[:ni, :ni] = np.clip(W[i, :ni, :ni].T * W_SCALE, -F8_MAX, F8_MAX)
        for vw, vcb, dw, dcb, pb, rows, rs in _SLOT_CHUNKS[j]:
            vimg = vblob[_VWIN_OFF[vw] : _VWIN_OFF[vw] + 128 * _VWIN_W[vw]]
            vimg = vimg.reshape(128, _VWIN_W[vw])
            vrl = V[rs : rs + rows]          # may be shorter than rows
            vimg[pb : pb + vrl.shape[0], vcb : vcb + NJ] = vrl.astype(vnp)
            dimg = dblob[_DWIN_OFF[dw] : _DWIN_OFF[dw] + 128 * _DWIN_W[dw]]
            dimg = dimg.reshape(128, _DWIN_W[dw])
            drl = Dt[rs : rs + rows]
            dimg[pb : pb + drl.shape[0], dcb : dcb + DCOL] = drl.astype(dnp)
    return vblob, dblob


def kernel(x, W, b):
    x = np.asarray(x, np.float32)
    W = np.asarray(W, np.float32)
    b = np.asarray(b, np.float32)

    nc = _build_program()
    in_maps = []
    for k in range(N_CORES):
        vb, db = _pack_core(k, x, W)
        in_maps.append({"vblob": vb, "dblob": db})
    res = run_bass_kernel_spmd(nc, in_maps, list(range(N_CORES)))

    y = x.copy()
    inv = 1.0 / W_SCALE
    for k in range(N_CORES):
        ob = res.results[k]["outblob"]
        for j in range(N_SLOTS):
            i = N_CORES * j + k
            ni = i + 1
            si, col = _SLOT_OUT[j]
            scw = _SG_CW[si]
            blk = np.asarray(
                ob[_SG_OFF[si] : _SG_OFF[si] + B * scw], np.float32
            ).reshape(B, scw)
            q = np.arange(ni)
            y[:, q, i - q] = blk[:, col : col + ni] * inv + b[i, :ni][None]
    return y


def emulate(x, W, b):
    """Exact host emulation of the device numeric path (for testing)."""
    x = np.asarray(x, np.float32)
    W = np.asarray(W, np.float32)
    b = np.asarray(b, np.float32)
    out_np = _np_dt(KCFG["out_dt"])
    y = x.copy()
    inv = 1.0 / W_SCALE
    for k in range(N_CORES):
        vb, db = _pack_core(k, x, W)
        for j in range(N_SLOTS):
            i = N_CORES * j + k
            ni = i + 1
            NJ = 8 * (j + 1)
            acc = np.zeros((B, NJ), np.float32)
            for vw, vcb, dw, dcb, pb, rows, _rs in _SLOT_CHUNKS[j]:
                vimg = vb[_VWIN_OFF[vw] : _VWIN_OFF[vw] + 128 * _VWIN_W[vw]]
                vimg = vimg.reshape(128, _VWIN_W[vw])
                dimg = db[_DWIN_OFF[dw] : _DWIN_OFF[dw] + 128 * _DWIN_W[dw]]
                dimg = dimg.reshape(128, _DWIN_W[dw])
                vch = vimg[pb : pb + rows, vcb : vcb + NJ].astype(np.float32)
                dch = dimg[pb : pb + rows, dcb : dcb + DCOL].astype(np.float32)
                acc += dch.T @ vch
            stg = acc.astype(out_np).astype(np.float32)
            q = np.arange(ni)
            y[:, q, i - q] = stg[:, :ni] * inv + b[i, :ni][None]
    return y


if __name__ == "__main__":
    vb = VBLOB_ELEMS * {"e3m4": 1, "e4m3": 1, "bf16": 2, "f32": 4}[KCFG["v_dt"]]
    db = DBLOB_ELEMS * {"bf16": 2, "f32": 4}[KCFG["d_dt"]]
    ob = OUT_ELEMS * {"f16": 2, "bf16": 2, "f32": 4}[KCFG["out_dt"]]
    print(f"V windows: {N_VWINS} ({_VWIN_W}), bytes {vb}")
    print(f"D windows: {N_DWINS} ({_DWIN_W}), bytes {db}")
    print(f"out bytes {ob}; total DMA {(vb + db + ob) / 1e6:.2f} MB")
    print(f"PE cols {sum(NJ for j in range(N_SLOTS) for NJ in [8 * (j + 1)] for _ in _SLOT_CHUNKS[j])}")
